# revision 14
# baseline (speedup 1.0000x reference)
"""Trainium2 Bass kernel for the attention-MLP problem.

Reference computation (S=32768, H=1024):
    cat    = [broadcast(hidden, (S, 2H)) | encoder_output]   # [S, 3H]
    energy = tanh(cat @ attn_w.T + attn_b)                   # [S, H]
    logits = (energy @ v_w.T).squeeze()                      # [S]
    out    = softmax(logits)                                 # [S]

Because the hidden rows are identical, cat @ attn_w.T splits into
    c0  = hidden @ W1T + attn_b          (one row, [H])
    pre = enc @ W2T + c0                  (the real work)
with W1T = attn_w[:, :2H].T and W2T = attn_w[:, 2H:].T.

Sharding: seq axis split across 8 cores (4096 rows each); weights
replicated. Softmax normalization uses exp (no max subtraction needed:
|logits| <= ||v_w||_1 ~ 26, safely inside fp32 exp range) with an
AllGather of the 8 per-core partial sums.

Per-core layout (OUT^T): encoder shard is shipped pre-transposed
[H, S_loc] in bf16 so the H contraction sits on SBUF partitions for both
matmul operands; psum tiles hold energy^T [j, s].

Schedule (v3). The kernel is PE-stream-bound: the main matmul issues
512 instructions x 512 moving columns per core per iteration, and the
measured column rate (~0.6 ns/col on this part, any dtype) puts its
floor at ~155 us; everything else is hidden under it.
  * bf16 operands (fp8 fails the 2e-2 tolerance: 7.1e-2 measured, and
    every residual-correction scheme costs >= bf16 time).
  * s-blocks processed 4 at a time (2 DMA pairs) per j-group; tanh over
    [128, 2, 512] psum tiles amortizes the ~352-cycle ACT ramp.
  * software-pipelined v-dot: the v^T @ tanh matmuls for group j are
    emitted after group j+1's main matmuls, so the PE never waits on
    ACT's tanh.
  * the 4 v-dots of a group land on PSUM quadrant rows 0/32/64/96 of
    one bank via tile_position, hitting 4 distinct PE column groups so
    they run concurrently (~18 us/iter faster than flat v-dots).
  * PSUM budget: 3x2 banks main chains + 1 bank logits + 1 bank c0 = 8.
"""

import numpy as np

import concourse.bass as bass
import concourse.mybir as mybir
import concourse.tile as tile
from concourse.bass_utils import run_bass_kernel_spmd

H = 1024
S = 32768
NCORES = 8
SL = S // NCORES          # 4096 rows per core
SB = 512                  # seq block (columns of the psum tiles)
NSB = SL // SB            # 8 seq blocks per core
NPAIR = NSB // 2          # 4 s-block pairs
KC = H // 128             # 8 contraction chunks
JC = H // 128             # 8 output-row chunks

F32 = mybir.dt.float32
F32R = mybir.dt.float32r
BF16 = mybir.dt.bfloat16

AF = mybir.ActivationFunctionType


# ---------------------------------------------------------------------------
# Workaround for this walrus build: instructions only accept a single
# sync-wait command, but Tile can attach several. Hoist the extra waits
# onto NOPs inserted just before the instruction on the same engine
# (engines execute their stream in order, so semantics are preserved).
def _split_multi_waits(nc):
    end_bb = nc.cur_bb.bb
    for bb in nc.m.functions[0].blocks:
        insts = list(bb.instructions)
        out = []
        changed = False
        for inst in insts:
            si = inst.sync_info
            waits = list(si.on_wait) if si and si.on_wait else []
            if len(waits) > 1:
                changed = True
                for w in waits[:-1]:
                    nop = nc.engines[inst.engine].nop(nofuse=True).ins
                    end_bb.instructions.remove(nop)
                    nop.sync_info = mybir.SyncInfo(on_wait=[w], on_update=[])
                    out.append(nop)
                si.on_wait = waits[-1:]
            out.append(inst)
        if changed:
            bb.instructions = out
# ---------------------------------------------------------------------------


def build(repeat: int = 1, main_dt: str = "bf16", single_core: bool = False,
          mode: str = "full"):
    """Build the per-core Bass module. `repeat` wraps the main compute in a
    For_i loop (used only by the benchmark harness to measure HW time by
    marginal wall-clock; the softmax tail + collective stay outside).
    mode: full | mm_only (perf experiment: main matmuls + dma only)."""
    mm_only = mode == "mm_only"
    MD = {"f32r": F32R, "bf16": BF16}[main_dt]
    nc = bass.Bass("TRN2", target_bir_lowering=False, debug=False,
                   num_devices=1 if single_core else NCORES)

    encT = nc.dram_tensor("encT", [H, SL], MD, kind="ExternalInput").ap()
    w2t = nc.dram_tensor("w2t", [H, H], MD, kind="ExternalInput").ap()
    w1t = nc.dram_tensor("w1t", [2 * H // NCORES, H], F32R,
                         kind="ExternalInput").ap()
    hidT = nc.dram_tensor("hidT", [128, 16 // NCORES], F32R,
                          kind="ExternalInput").ap()
    bias = nc.dram_tensor("bias", [1, H], F32, kind="ExternalInput").ap()
    vwc = nc.dram_tensor("vwc", [128, JC], BF16, kind="ExternalInput").ap()
    out = nc.dram_tensor("out", [1, SL], F32, kind="ExternalOutput").ap()

    encT_v = encT.rearrange("(k p) s -> p k s", p=128)   # [128, 8, 4096]
    w2t_v = w2t.rearrange("(k p) j -> p k j", p=128)     # [128, 8, 1024]
    w1t_v = w1t.rearrange("(k p) j -> p k j", p=128)     # [128, 2, 1024]

    with tile.TileContext(nc) as tc:
        with (
            tc.tile_pool(name="const", bufs=1) as const_pool,
            tc.tile_pool(name="enc", bufs=4) as enc_pool,
            tc.tile_pool(name="tanh", bufs=4) as tanh_pool,
            tc.tile_pool(name="sm", bufs=1) as sm_pool,
            tc.tile_pool(name="pse", bufs=3, space="PSUM") as pse_pool,
            tc.tile_pool(name="psa", bufs=1, space="PSUM") as psa_pool,
            tc.tile_pool(name="psc", bufs=1, space="PSUM") as psc_pool,
            tc.tile_pool(name="dram", bufs=1, space="DRAM") as dram_pool,
        ):
            # --- tiny constants -------------------------------------------
            hid_sb = const_pool.tile([128, 16 // NCORES], F32R)
            nc.sync.dma_start(hid_sb[:], hidT[:])
            vw_sb = const_pool.tile([128, JC], BF16)
            nc.sync.dma_start(vw_sb[:], vwc[:])
            b_sb = const_pool.tile([1, H], F32)
            nc.sync.dma_start(b_sb[:], bias[:])

            # --- replicated weights: one tile per j-slab so the group-j
            # matmuls depend only on their own slab's DMA ---------------
            w2_tiles = []
            for j in range(JC):
                w2_j = const_pool.tile([128, KC, 128], MD, name=f"w2_{j}")
                nc.sync.dma_start(w2_j[:], w2t_v[:, :, j * 128:(j + 1) * 128])
                w2_tiles.append(w2_j)

            exps = sm_pool.tile([1, SL], F32)
            sums = sm_pool.tile([1, NSB], F32)

            # --- c0 = hidden @ W1T + attn_b (one row), sharded over cores
            c0_sb = const_pool.tile([128, JC], F32)

            NKC = 16 // NCORES   # local w1 chunks (c0 sharded over cores)

            def c0_section():
                w1_sb = const_pool.tile([128, NKC, H], F32R)
                nc.sync.dma_start(w1_sb[:], w1t_v[:])
                # bias arrives pre-divided by NCORES, so adding it to the
                # local partial and AllReduce-summing reconstructs c0+b
                part_row = const_pool.tile([1, H], F32)
                for half in range(2):
                    psum_c = psc_pool.tile([1, 512], F32, tag="c0ps",
                                           name="psum_c")
                    for kc in range(NKC):
                        nc.tensor.matmul(
                            psum_c[:],
                            hid_sb[:, kc:kc + 1],
                            w1_sb[:, kc, half * 512:(half + 1) * 512],
                            start=(kc == 0), stop=(kc == NKC - 1),
                        )
                    nc.vector.tensor_add(
                        part_row[:, half * 512:(half + 1) * 512],
                        psum_c[:],
                        b_sb[:, half * 512:(half + 1) * 512])
                ar_in = dram_pool.tile([1, H], F32)
                nc.gpsimd.dma_start(ar_in[:], part_row[:])
                if single_core:
                    ar_out = ar_in
                else:
                    ar_out = dram_pool.tile([1, H], F32)
                    nc.gpsimd.collective_compute(
                        "AllReduce",
                        mybir.AluOpType.add,
                        replica_groups=[list(range(NCORES))],
                        ins=[ar_in.opt()],
                        outs=[ar_out.opt()],
                    )
                nc.sync.dma_start(
                    c0_sb[:],
                    ar_out[:].rearrange("o (j p) -> (o p) j", p=128)
                )

            # --- main pipeline -------------------------------------------
            def main_body(_iv=None):
                # halves of 4 s-blocks; one [128, SB] psum_a bank whose
                # quadrant rows 0/32/64/96 hold the 4 s-blocks' logits so
                # the 4 v-dots of a group land on distinct PE column
                # groups and run concurrently.
                psum_a = [None]
                pending = []               # delayed v-dot emissions

                def flush():
                    for emit in pending:
                        emit()
                    pending.clear()

                def make_vdot(j, th2s, pa):
                    def emit():
                        for q in range(4):
                            r = 32 * q
                            nc.tensor.matmul(
                                pa[r:r + 1, :],
                                vw_sb[:, j:j + 1], th2s[q // 2][:, q % 2, :],
                                tile_position=(0, r),
                                start=(j == 0), stop=(j == JC - 1),
                            )
                    return emit

                def emit_exps(h, pa):
                    for q in range(4):
                        sb = 4 * h + q
                        nc.scalar.activation(
                            exps[:, sb * SB:(sb + 1) * SB],
                            pa[32 * q:32 * q + 1, :], AF.Exp,
                            accum_out=sums[:, sb:sb + 1],
                        )

                prev_pa = None
                for h in range(2):
                    enc_ts = []
                    for pp in range(2):     # two s-block pairs per half
                        enc_t = enc_pool.tile([128, KC, 2, SB], MD,
                                              tag="enc")
                        o = (4 * h + 2 * pp) * SB
                        nc.sync.dma_start(
                            enc_t[:],
                            encT_v[:, :, o:o + 2 * SB]
                            .rearrange("p k (i s) -> p k i s", i=2),
                        )
                        enc_ts.append(enc_t)
                    for j in range(JC):
                        pe2s = []
                        for pp in range(2):
                            pe2 = pse_pool.tile([128, 2, SB], F32,
                                                tag="pe2", name="pe2")
                            for k in range(KC):
                                w = w2_tiles[j][:, k, :]
                                for i in range(2):
                                    nc.tensor.matmul(
                                        pe2[:, i, :], w,
                                        enc_ts[pp][:, k, i, :],
                                        start=(k == 0), stop=(k == KC - 1),
                                    )
                            pe2s.append(pe2)
                        if mm_only:
                            continue
                        flush()
                        if j == 0:
                            # previous half's logits complete: exp them
                            # before this half's first v-dots reuse the bank
                            if h == 1:
                                emit_exps(0, prev_pa)
                            psum_a[0] = psa_pool.tile(
                                [128, SB], F32, tag="psa", name="psa")
                        th2s = []
                        for pp in range(2):
                            th2 = tanh_pool.tile([128, 2, SB], BF16,
                                                 tag="th2", name="th2")
                            nc.scalar.activation(
                                th2[:], pe2s[pp][:], AF.Tanh,
                                bias=c0_sb[:, j:j + 1])
                            th2s.append(th2)
                        pending.append(make_vdot(j, th2s, psum_a[0]))
                    prev_pa = psum_a[0]
                if not mm_only:
                    flush()
                    emit_exps(1, prev_pa)
                else:
                    nc.gpsimd.memset(exps[:], 1.0)
                    nc.gpsimd.memset(sums[:], 1.0)

            c0_section()
            if repeat == 1:
                main_body()
            else:
                with tc.For_i(0, repeat, 1,
                              hint_engines=(mybir.EngineType.PE,)) as _i:
                    main_body(_i)

            # --- softmax normalization across cores -----------------------
            if single_core:
                zg = sm_pool.tile([1, 1], F32)
                nc.vector.reduce_sum(zg[:], sums[:],
                                     axis=mybir.AxisListType.X)
            else:
                # AllGather the raw per-block sums (8 floats/core) and do a
                # single 64-element reduce afterwards
                ag_in = dram_pool.tile([1, NSB], F32)
                nc.gpsimd.dma_start(ag_in[:], sums[:])
                ag_out = dram_pool.tile([1, NCORES * NSB], F32)
                nc.gpsimd.collective_compute(
                    "AllGather",
                    mybir.AluOpType.bypass,
                    replica_groups=[list(range(NCORES))],
                    ins=[ag_in.opt()],
                    outs=[ag_out.opt()],
                )
                zs = sm_pool.tile([1, NCORES * NSB], F32)
                nc.gpsimd.dma_start(zs[:], ag_out[:])
                zg = sm_pool.tile([1, 1], F32)
                nc.vector.reduce_sum(zg[:], zs[:], axis=mybir.AxisListType.X)
            invz = sm_pool.tile([1, 1], F32)
            nc.vector.reciprocal(invz[:], zg[:])
            outv = sm_pool.tile([1, SL], F32)
            # split the 4096-element scale across ACT and DVE in parallel,
            # and ship each half as soon as it's done
            hl = SL // 2
            nc.scalar.activation(outv[:, :hl], exps[:, :hl], AF.Identity,
                                 scale=invz[:])
            nc.sync.dma_start(out[:, :hl], outv[:, :hl])
            nc.vector.tensor_scalar_mul(outv[:, hl:], exps[:, hl:], invz[:])
            nc.sync.dma_start(out[:, hl:], outv[:, hl:])

    _split_multi_waits(nc)
    return nc


def prepare_in_maps(hidden, encoder_output, attn_w, attn_b, v_w,
                    main_dt="bf16"):
    hidden = np.asarray(hidden, dtype=np.float32)
    enc = np.asarray(encoder_output, dtype=np.float32)
    attn_w = np.asarray(attn_w, dtype=np.float32)
    attn_b = np.asarray(attn_b, dtype=np.float32)
    v_w = np.asarray(v_w, dtype=np.float32)

    import ml_dtypes
    md = np.float32 if main_dt == "f32r" else ml_dtypes.bfloat16
    w2t = np.ascontiguousarray(attn_w[:, 2 * H:].T).astype(md)   # [H, H]
    w1t_full = np.ascontiguousarray(attn_w[:, :2 * H].T)
    hidT_full = np.ascontiguousarray(hidden.reshape(16, 128).T)
    kpc = 16 // NCORES
    b = np.ascontiguousarray(attn_b.reshape(1, H)) / np.float32(NCORES)
    vwc = np.ascontiguousarray(v_w.reshape(JC, 128).T).astype(
        ml_dtypes.bfloat16)  # [128, 8]

    in_maps = []
    for c in range(NCORES):
        encT = np.ascontiguousarray(enc[c * SL:(c + 1) * SL, :].T).astype(md)
        in_maps.append({
            "encT": encT, "w2t": w2t,
            "w1t": np.ascontiguousarray(
                w1t_full[c * kpc * 128:(c + 1) * kpc * 128, :]),
            "hidT": np.ascontiguousarray(
                hidT_full[:, c * kpc:(c + 1) * kpc]),
            "bias": b, "vwc": vwc,
        })
    return in_maps


_NC_CACHE = {}


def _get_nc(repeat: int = 1):
    if repeat not in _NC_CACHE:
        _NC_CACHE[repeat] = build(repeat)
    return _NC_CACHE[repeat]


def kernel(hidden, encoder_output, attn_w, attn_b, v_w):
    nc = _get_nc(1)
    in_maps = prepare_in_maps(hidden, encoder_output, attn_w, attn_b, v_w)
    res = run_bass_kernel_spmd(nc, in_maps, list(range(NCORES)))
    return np.concatenate([res.results[c]["out"][0] for c in range(NCORES)])
